# revision 2
# baseline (speedup 1.0000x reference)
"""Trainium2 Bass kernel for nn_DeepRNN: 4-layer tanh RNN, H=1024, T=256, B=64.

Strategy (V1): layer-sequential passes on every core (replicated), full batch.
Each layer pass runs the T-step recurrence with the input matmul, bias, and
recurrent matmul folded into one PSUM accumulation group per step:

    pre[t] = [inp[t]; 1; h[t-1]] @ [Wx; bh; Wh]      (fp32r matmuls)
    h[t]   = tanh(pre[t])

Layout: hidden state lives transposed ([H, B] chunk tiles) so it feeds the
next step's stationary operand directly; batch=64 halves are col-tiled onto
PSUM partitions 0:63 / 64:127 so one [128, 512] PSUM bank holds the whole
[64, 1024] pre-activation. Layer outputs stream through DRAM ping-pong
buffers; only the final hidden state feeds the classifier head.
"""
import sys
sys.path.insert(0, "/opt/trn_rl_repo")

import numpy as np
import concourse.bacc as bacc
import concourse.mybir as mybir
import concourse.tile as tile
from concourse.bass_utils import run_bass_kernel_spmd
try:
    from runner import run_cached
except ImportError:
    run_cached = None

FP32 = mybir.dt.float32
FP32R = mybir.dt.float32r
TANH = mybir.ActivationFunctionType.Tanh

N_CORES = 8
I, H, L, C, B, T = 256, 1024, 4, 1000, 64, 256
HC = H // 128           # 8 hidden chunks
IC = I // 128           # 2 input chunks (layer 0)
BLK = 8                 # recurrence steps per DMA block


def _r(ap):
    return ap


def _emit_step(nc, psA, psB, wt, in_lhsTs, ones_row, h_lhsTs):
    """One recurrence step's PSUM accumulation: input chunks + bias + hidden.

    fp32r matmuls must write PSUM at partition base 0, so the two 512-wide
    halves of the [64, 1024] pre-activation accumulate in two separate banks.
    """
    n_in = len(in_lhsTs)
    nmm = n_in + 1 + len(h_lhsTs)
    idx = 0
    for lhsT_list, kbase in ((in_lhsTs, 0), ([ones_row], n_in),
                             (h_lhsTs, n_in + 1)):
        for j, lhsT in enumerate(lhsT_list):
            k = kbase + j
            st, sp = idx == 0, idx == nmm - 1
            nc.tensor.matmul(psA[:], lhsT, wt[:, k, 0:512], start=st, stop=sp)
            nc.tensor.matmul(psB[:], lhsT, wt[:, k, 512:1024], start=st, stop=sp)
            idx += 1


def build(T_steps=T, repeat=1):
    nc = bacc.Bacc()
    n_blocks = T_steps // BLK

    # ---- parameters (per-core identical in V1) ----
    xT = nc.declare_dram_parameter("xT", [128, T_steps, IC, B], FP32R, isOutput=False)
    Wl = [nc.declare_dram_parameter(f"W{l}", [(IC if l == 0 else HC) + 1 + HC, 128, H],
                                    FP32R, isOutput=False) for l in range(L)]
    Wo_p = nc.declare_dram_parameter("Wo", [HC, 128, C], FP32R, isOutput=False)
    Wob = nc.declare_dram_parameter("Wob", [128, C], FP32R, isOutput=False)
    ident = nc.declare_dram_parameter("ident", [128, B], FP32, isOutput=False)
    onesr = nc.declare_dram_parameter("onesr", [128, B], FP32R, isOutput=False)
    out = nc.declare_dram_parameter("out", [B, C], FP32, isOutput=True)

    with tile.TileContext(nc) as tc:
        with (
            tc.tile_pool(name="wpool", bufs=1) as wpool,
            tc.tile_pool(name="iopool", bufs=3) as iopool,
            tc.tile_pool(name="hpool", bufs=2) as hpool,
            tc.tile_pool(name="cpool", bufs=1) as cpool,
            tc.tile_pool(name="pspool", bufs=2, space="PSUM") as pspool,
            tc.tile_pool(name="ptpool", bufs=2, space="PSUM") as ptpool,
            tc.tile_pool(name="dpool", bufs=1, space="DRAM") as dpool,
        ):
            ident_sb = cpool.tile([128, B], FP32, tag="ident")
            ones_sb = cpool.tile([128, B], FP32R, tag="ones")
            nc.sync.dma_start(ident_sb[:], ident[:])
            nc.sync.dma_start(ones_sb[:], onesr[:])

            hstream = [dpool.tile([128, T_steps, HC, B], FP32R, tag=f"hs{i}",
                                  name=f"hs{i}")
                       for i in range(2)]

            hT_last = None
            for _rep in range(repeat):
              for l in range(L):
                  in_ch = IC if l == 0 else HC
                  nch = in_ch + 1 + HC
                  wt = wpool.tile([128, nch, H], FP32R, tag="W")
                  nc.sync.dma_start(wt[:], Wl[l].rearrange("k p n -> p k n"))

                  in_stream = xT if l == 0 else hstream[(l - 1) % 2]
                  out_stream = hstream[l % 2]

                  hT_prev = None  # AP of [128, HC, B] hidden state chunks
                  for blk in range(n_blocks):
                      inb = iopool.tile([128, BLK, in_ch, B], FP32R, tag="inb")
                      nc.sync.dma_start(
                          inb[:], in_stream[:, blk * BLK:(blk + 1) * BLK, :, :])
                      outb = iopool.tile([128, BLK, HC, B], FP32R, tag="outb")
                      for tr in range(BLK):
                          psA = pspool.tile([64, 512], FP32, tag="psA")
                          psB = pspool.tile([64, 512], FP32, tag="psB")
                          in_lhsTs = [inb[:, tr, c, :] for c in range(in_ch)]
                          h_lhsTs = ([] if hT_prev is None else
                                     [hT_prev[:, c, :] for c in range(HC)])
                          _emit_step(nc, psA, psB, wt, in_lhsTs, ones_sb, h_lhsTs)

                          h_sb = hpool.tile([64, H], FP32, tag="h")
                          nc.scalar.activation(h_sb[:, 0:512], psA[:], TANH)
                          nc.scalar.activation(h_sb[:, 512:H], psB[:], TANH)

                          pt = ptpool.tile([128, HC, B], FP32, tag="pt")
                          for c in range(HC):
                              nc.tensor.transpose(pt[:, c, :],
                                                  h_sb[:, c * 128:(c + 1) * 128],
                                                  ident_sb[0:64, 0:64])
                          nc.vector.tensor_copy(outb[:, tr, :, :], pt[:])
                          hT_prev = outb[:, tr, :, :]
                      if l < L - 1:
                          nc.sync.dma_start(
                              out_stream[:, blk * BLK:(blk + 1) * BLK, :, :], outb[:])
                  hT_last = hT_prev

            # ---- classifier head: out = h3[T] @ Wo + bo ----
            wo_sb = wpool.tile([128, HC, C], FP32R, tag="W")
            nc.sync.dma_start(wo_sb[:], Wo_p.rearrange("k p n -> p k n"))
            wob_sb = cpool.tile([128, C], FP32R, tag="wob")
            nc.sync.dma_start(wob_sb[:], Wob[:])

            pA = pspool.tile([64, 512], FP32, tag="psA")
            pB = pspool.tile([64, C - 512], FP32, tag="psB")
            for c in range(HC + 1):
                lhsT = ones_sb[:, 0:64] if c == HC else hT_last[:, c, :]
                rhs = wob_sb if c == HC else wo_sb[:, c, :]
                st, sp = c == 0, c == HC
                nc.tensor.matmul(pA[:], _r(lhsT), _r(rhs[:, 0:512]),
                                 start=st, stop=sp)
                nc.tensor.matmul(pB[:], _r(lhsT), _r(rhs[:, 512:C]),
                                 start=st, stop=sp)
            out_sb = hpool.tile([64, C], FP32, tag="osb")
            nc.scalar.activation(out_sb[:, 0:512], pA[:],
                                 mybir.ActivationFunctionType.Copy)
            nc.scalar.activation(out_sb[:, 512:C], pB[:],
                                 mybir.ActivationFunctionType.Copy)
            nc.sync.dma_start(out[:], out_sb[:])

    nc.compile()
    return nc


def _pack_inputs(x, Wx0, Wx, Wh, bh, Wo, bo):
    T_steps = x.shape[1]
    f32 = np.float32
    # xT[p, t, c, b] = x[b, t, c*128+p]
    xT = np.ascontiguousarray(
        x.transpose(2, 1, 0).reshape(IC, 128, T_steps, B).transpose(1, 2, 0, 3),
        dtype=f32)
    Ws = {}
    for l in range(L):
        bias_block = np.zeros((128, H), f32)
        bias_block[0] = bh[l]
        wx = Wx0 if l == 0 else Wx[l - 1]
        wfull = np.concatenate([wx, bias_block, Wh[l]], axis=0)
        Ws[f"W{l}"] = np.ascontiguousarray(
            wfull.reshape(-1, 128, H), dtype=f32)
    wob = np.zeros((128, C), f32)
    wob[0] = bo
    eye = np.eye(64, dtype=f32)
    return {
        "xT": xT, **Ws,
        "Wo": np.ascontiguousarray(Wo.reshape(HC, 128, C), dtype=f32),
        "Wob": wob,
        "ident": np.ascontiguousarray(np.vstack([eye, eye])),
        "onesr": np.vstack([np.ones((1, B), f32), np.zeros((127, B), f32)]),
    }


_BUILT = {}


def kernel(x, Wx0, Wx, Wh, bh, Wo, bo, _trace=False):
    T_steps = x.shape[1]
    if T_steps not in _BUILT:
        _BUILT[T_steps] = build(T_steps)
    nc = _BUILT[T_steps]
    in_map = _pack_inputs(np.asarray(x, np.float32), np.asarray(Wx0, np.float32),
                          np.asarray(Wx, np.float32), np.asarray(Wh, np.float32),
                          np.asarray(bh, np.float32), np.asarray(Wo, np.float32),
                          np.asarray(bo, np.float32))
    in_maps = [in_map] * N_CORES
    res = run_bass_kernel_spmd(nc, in_maps, list(range(N_CORES)), trace=_trace)
    kernel.last_results = res
    return res.results[0]["out"]


if __name__ == "__main__":
    # quick self-test at reduced T against a numpy reference
    Tt = int(sys.argv[1]) if len(sys.argv) > 1 else 32
    rng = np.random.default_rng(0)
    STDV = 1.0 / np.sqrt(H)
    u = lambda *s: rng.uniform(-STDV, STDV, s).astype(np.float32)
    x = rng.standard_normal((B, Tt, I), dtype=np.float32)
    Wx0, Wx_, Wh_ = u(I, H), u(L - 1, H, H), u(L, H, H)
    bh_, Wo_, bo_ = u(L, H), u(H, C), u(C)

    h = np.zeros((L, B, H), np.float32)
    for t in range(Tt):
        inp = x[:, t, :]
        for l in range(L):
            pre = inp @ (Wx0 if l == 0 else Wx_[l - 1]) + h[l] @ Wh_[l] + bh_[l]
            h[l] = np.tanh(pre)
            inp = h[l]
    expected = h[-1] @ Wo_ + bo_

    got = kernel(x, Wx0, Wx_, Wh_, bh_, Wo_, bo_)
    err = np.abs(got - expected).max() / np.abs(expected).max()
    print(f"T={Tt}  max-rel-err: {err:.3e}")



# revision 3
# speedup vs baseline: 1.3745x; 1.3745x over previous
"""Trainium2 Bass kernel for nn_DeepRNN — V2: 4-stage layer pipeline.

Topology: 8 cores = 2 independent groups of 4 (replica groups [[0..3],[4..7]]).
Within a group, core l owns layer l. Uniform SPMD instruction stream; per-core
behavior differs only through input data (weights, masks, gates).

Per slot s (time-block of BLK steps), every core:
  1. DMAs its x block (layer-0 data; ignored elsewhere via zero weights).
  2. Reads the AllGather output of slot s-2, selects the previous core's h
     block with a per-core one-hot mask (4 fused mul-add ops on DVE).
  3. Batched input GEMM (bf16): xpart[t] = h_in[t] @ Wx + x[t] @ Zx + bh
     for all BLK steps at once (PSUM -> sbuf bf16).
  4. Recurrence (fp32r): for each step, PSUM is prefilled with xpart via
     the Act engine, then 16 accumulating matmuls add h[t-1] @ Wh, then
     tanh -> h, PE-transpose -> h^T (stationary for the next step) and a
     bf16 copy into the send staging tile.
  5. DMAs the staged h^T block to a DRAM bounce buffer and issues an
     AllGather so the next core can consume it at slot s+2.

Pipeline skew is 2 slots/stage so collectives fully overlap compute.
Core c processes block b at slot b + 2*(c%4); per-(core,slot) gate inputs
zero the recurrent carry at each core's first real block. Core 3 (and 7)
ends with the real h_3[T-1]; the classifier head runs on every core and
the host reads core 3's output.
"""
import sys
sys.path.insert(0, "/opt/trn_rl_repo")

import numpy as np
import ml_dtypes
import concourse.bacc as bacc
import concourse.mybir as mybir
import concourse.tile as tile
from concourse.bass_utils import run_bass_kernel_spmd

FP32 = mybir.dt.float32
FP32R = mybir.dt.float32r
BF16 = mybir.dt.bfloat16
TANH = mybir.ActivationFunctionType.Tanh
COPY = mybir.ActivationFunctionType.Copy
ADD = mybir.AluOpType.add
MULT = mybir.AluOpType.mult
BYPASS = mybir.AluOpType.bypass

N_CORES = 8
I, H, L, C, B, T = 256, 1024, 4, 1000, 64, 256
HC = H // 128            # 8 hidden chunks
IC = I // 128            # 2 input chunks
GK = HC + IC + 1         # gemm K-chunks: h, x, bias
BLK = 8                  # steps per block
SKEW = 2                 # slots of pipeline lag per stage
GROUPS = [[0, 1, 2, 3], [4, 5, 6, 7]]


def build(T_steps=T):
    nc = bacc.Bacc(num_devices=N_CORES)
    NB = T_steps // BLK
    NSLOT = NB + SKEW * (L - 1)

    Wh = nc.declare_dram_parameter("Wh", [128, HC, H], FP32R, isOutput=False)
    Wg = nc.declare_dram_parameter("Wg", [128, GK, H], BF16, isOutput=False)
    xT = nc.declare_dram_parameter("xT", [128, NB, IC, BLK, B], BF16,
                                   isOutput=False)
    hmask = nc.declare_dram_parameter("hmask", [128, 4], FP32, isOutput=False)
    gates = nc.declare_dram_parameter("gates", [128, NSLOT], FP32,
                                      isOutput=False)
    ident = nc.declare_dram_parameter("ident", [128, B], FP32, isOutput=False)
    onesg = nc.declare_dram_parameter("onesg", [128, B], BF16, isOutput=False)
    hzero = nc.declare_dram_parameter("hzero", [128, HC, B], FP32R,
                                      isOutput=False)
    Wo_p = nc.declare_dram_parameter("Wo", [128, HC, C], BF16, isOutput=False)
    Wob = nc.declare_dram_parameter("Wob", [128, C], BF16, isOutput=False)
    out = nc.declare_dram_parameter("out", [B, C], FP32, isOutput=True)

    with tile.TileContext(nc) as tc:
        with (
            tc.tile_pool(name="wpool", bufs=1) as wpool,
            tc.tile_pool(name="cpool", bufs=1) as cpool,
            tc.tile_pool(name="xpool", bufs=2) as xpool,
            tc.tile_pool(name="agpool", bufs=1) as agpool,
            tc.tile_pool(name="tpool", bufs=1) as tpool,
            tc.tile_pool(name="hpool", bufs=2) as hpool,
            tc.tile_pool(name="xppool", bufs=2) as xppool,
            tc.tile_pool(name="stpool", bufs=2) as stpool,
            tc.tile_pool(name="htpool", bufs=3) as htpool,
            tc.tile_pool(name="hspool", bufs=2) as hspool,
            tc.tile_pool(name="ps_rec", bufs=2, space="PSUM") as ps_rec,
            tc.tile_pool(name="ps_tr", bufs=2, space="PSUM") as ps_tr,
            tc.tile_pool(name="dpool", bufs=1, space="DRAM") as dpool,
        ):
            wh_sb = wpool.tile([128, HC, H], FP32R, tag="wh")
            nc.sync.dma_start(wh_sb[:], Wh[:])
            wg_sb = wpool.tile([128, GK, H], BF16, tag="wg")
            nc.sync.dma_start(wg_sb[:], Wg[:])
            mask_sb = cpool.tile([128, 4], FP32, tag="hm")
            nc.sync.dma_start(mask_sb[:], hmask[:])
            gates_sb = cpool.tile([128, NSLOT], FP32, tag="gt")
            nc.sync.dma_start(gates_sb[:], gates[:])
            ident_sb = cpool.tile([128, B], FP32, tag="id")
            nc.sync.dma_start(ident_sb[:], ident[:])
            onesg_sb = cpool.tile([128, B], BF16, tag="og")
            nc.sync.dma_start(onesg_sb[:], onesg[:])

            hT_init = cpool.tile([128, HC, B], FP32R, tag="h0")
            nc.sync.dma_start(hT_init[:], hzero[:])

            bin_t = [dpool.tile([128, BLK, HC, B], BF16, tag=f"bi{i}",
                                name=f"bi{i}") for i in range(2)]
            bout_t = [dpool.tile([4, 128, BLK, HC, B], BF16, tag=f"bo{i}",
                                 name=f"bo{i}") for i in range(2)]

            hT_prev = hT_init
            stage = None
            for s in range(NSLOT):
                bx = min(s, NB - 1)
                xb = xpool.tile([128, IC, BLK, B], BF16, tag="xb")
                nc.sync.dma_start(xb[:], xT[:, bx, :, :, :])

                hin = None
                if s >= SKEW:
                    bout = bout_t[(s - SKEW) % 2]
                    acc = [tpool.tile([128, BLK, HC, B], BF16, tag="hinA",
                                      name="hinA"),
                           hpool.tile([128, BLK, HC, B], BF16, tag="hinB",
                                      name="hinB")]
                    for j in range(4):
                        ag = agpool.tile([128, BLK, HC, B], BF16,
                                         tag=f"ag{j % 2}")
                        nc.sync.dma_start(ag[:], bout[j])
                        if j == 0:
                            nc.vector.tensor_scalar_mul(
                                acc[0][:], ag[:], mask_sb[:, 0:1])
                        else:
                            # acc_new = (ag * mask_j) + acc_old
                            nc.vector.scalar_tensor_tensor(
                                acc[j % 2][:], ag[:], mask_sb[:, j:j + 1],
                                acc[(j - 1) % 2][:], op0=MULT, op1=ADD)
                    hin = acc[3 % 2]

                # ---- batched input GEMM for this slot's BLK steps ----
                xpart = xppool.tile([64, BLK, H], BF16, tag="xp")
                for t in range(BLK):
                    gA = ps_rec.tile([64, 512], FP32, tag="pA")
                    gB = ps_rec.tile([64, 512], FP32, tag="pB")
                    chunks = []
                    if hin is not None:
                        chunks += [(hin[:, t, k, :], k) for k in range(HC)]
                    chunks += [(xb[:, c, t, :], HC + c) for c in range(IC)]
                    chunks.append((onesg_sb[0:1, :], HC + IC))
                    n = len(chunks)
                    for i, (lhsT, kidx) in enumerate(chunks):
                        st, sp = i == 0, i == n - 1
                        kp = 1 if kidx == HC + IC else 128
                        nc.tensor.matmul(gA[:], lhsT, wg_sb[0:kp, kidx, 0:512],
                                         start=st, stop=sp)
                        nc.tensor.matmul(gB[:], lhsT, wg_sb[0:kp, kidx, 512:H],
                                         start=st, stop=sp)
                    nc.scalar.activation(xpart[:, t, 0:512], gA[:], COPY)
                    nc.scalar.activation(xpart[:, t, 512:H], gB[:], COPY)

                # ---- recurrence over the BLK steps ----
                hT_g = htpool.tile([128, HC, B], FP32R, tag="hg")
                nc.vector.tensor_scalar_mul(hT_g[:], hT_prev[:],
                                            gates_sb[:, s:s + 1])
                hT_prev = hT_g
                stage = stpool.tile([128, BLK, HC, B], BF16, tag="st")
                for t in range(BLK):
                    pA = ps_rec.tile([64, 512], FP32, tag="pA")
                    pB = ps_rec.tile([64, 512], FP32, tag="pB")
                    nc.scalar.activation(pA[:], xpart[:, t, 0:512], COPY)
                    nc.scalar.activation(pB[:], xpart[:, t, 512:H], COPY)
                    for k in range(HC):
                        sp = k == HC - 1
                        nc.tensor.matmul(pA[:], hT_prev[:, k, :],
                                         wh_sb[:, k, 0:512], start=False,
                                         stop=sp, skip_group_check=True)
                        nc.tensor.matmul(pB[:], hT_prev[:, k, :],
                                         wh_sb[:, k, 512:H], start=False,
                                         stop=sp, skip_group_check=True)
                    h_sb = hspool.tile([64, H], FP32, tag="h")
                    nc.scalar.activation(h_sb[:, 0:512], pA[:], TANH)
                    nc.scalar.activation(h_sb[:, 512:H], pB[:], TANH)
                    pt = ps_tr.tile([128, HC, B], FP32, tag="pt")
                    for k in range(HC):
                        nc.tensor.transpose(pt[:, k, :],
                                            h_sb[:, k * 128:(k + 1) * 128],
                                            ident_sb[0:64, 0:64])
                    hT_new = htpool.tile([128, HC, B], FP32R, tag="ht")
                    nc.vector.tensor_copy(hT_new[:], pt[:])
                    nc.vector.tensor_copy(stage[:, t, :, :], pt[:])
                    hT_prev = hT_new

                # ---- publish h block ----
                bin_ = bin_t[s % 2]
                nc.sync.dma_start(bin_[:], stage[:])
                nc.gpsimd.collective_compute(
                    "AllGather", BYPASS, replica_groups=GROUPS,
                    ins=[bin_[:].opt()], outs=[bout_t[s % 2][:].opt()])

            # ---- classifier head: out = h3[T-1] @ Wo + bo (bf16) ----
            wo_sb = wpool.tile([128, HC, C], BF16, tag="wg")
            nc.sync.dma_start(wo_sb[:], Wo_p[:])
            wob_sb = cpool.tile([128, C], BF16, tag="wob")
            nc.sync.dma_start(wob_sb[:], Wob[:])

            hA = ps_rec.tile([64, 512], FP32, tag="pA")
            hB = ps_rec.tile([64, 512], FP32, tag="pB")
            hT_bf = stage[:, BLK - 1, :, :]
            for k in range(HC + 1):
                st, sp = k == 0, k == HC
                if k == HC:
                    nc.tensor.matmul(hA[:], onesg_sb[0:1, :],
                                     wob_sb[0:1, 0:512], start=st, stop=sp)
                    nc.tensor.matmul(hB[:, 0:C - 512], onesg_sb[0:1, :],
                                     wob_sb[0:1, 512:C], start=st, stop=sp)
                else:
                    nc.tensor.matmul(hA[:], hT_bf[:, k, :],
                                     wo_sb[:, k, 0:512], start=st, stop=sp)
                    nc.tensor.matmul(hB[:, 0:C - 512], hT_bf[:, k, :],
                                     wo_sb[:, k, 512:C], start=st, stop=sp)
            out_sb = hspool.tile([64, C], FP32, tag="osb")
            nc.scalar.activation(out_sb[:, 0:512], hA[:], COPY)
            nc.scalar.activation(out_sb[:, 512:C], hB[:, 0:C - 512], COPY)
            nc.sync.dma_start(out[:], out_sb[:])

    nc.compile()
    return nc, NSLOT


def _to_bf16(a):
    return np.asarray(a, dtype=ml_dtypes.bfloat16)


def _pack_core(c, x, Wx0, Wx, Wh_, bh, Wo, bo, NB, NSLOT):
    l = c % 4
    f32 = np.float32
    T_steps = NB * BLK

    wh = np.ascontiguousarray(
        Wh_[l].reshape(HC, 128, H).transpose(1, 0, 2), dtype=f32)

    wg = np.zeros((128, GK, H), f32)
    if l > 0:
        wg[:, 0:HC, :] = Wx[l - 1].reshape(HC, 128, H).transpose(1, 0, 2)
    else:
        wg[:, HC:HC + IC, :] = Wx0.reshape(IC, 128, H).transpose(1, 0, 2)
    wg[0, HC + IC, :] = bh[l]

    # xT[p, blk, c, t8, b] = x[b, blk*BLK+t8, c*128+p]
    xt = x.transpose(2, 1, 0).reshape(IC, 128, NB, BLK, B).transpose(
        1, 2, 0, 3, 4)

    hm = np.zeros((128, 4), f32)
    if l > 0:
        hm[:, l - 1] = 1.0

    gt = np.zeros((128, NSLOT), f32)
    gt[:, :] = (np.arange(NSLOT) > 2 * l).astype(f32)[None, :]

    eye = np.eye(64, dtype=f32)
    ones = np.zeros((128, B), f32)
    ones[0] = 1.0

    wob = np.zeros((128, C), f32)
    wob[0] = bo

    return {
        "Wh": wh,
        "Wg": _to_bf16(wg),
        "xT": _to_bf16(np.ascontiguousarray(xt)),
        "hmask": hm,
        "gates": gt,
        "ident": np.ascontiguousarray(np.vstack([eye, eye])),
        "onesg": _to_bf16(ones),
        "hzero": np.zeros((128, HC, B), f32),
        "Wo": _to_bf16(Wo.reshape(HC, 128, C).transpose(1, 0, 2)),
        "Wob": _to_bf16(wob),
    }


_BUILT = {}


def kernel(x, Wx0, Wx, Wh, bh, Wo, bo, _trace=False):
    T_steps = x.shape[1]
    NB = T_steps // BLK
    if T_steps not in _BUILT:
        _BUILT[T_steps] = build(T_steps)
    nc, NSLOT = _BUILT[T_steps]
    args = [np.asarray(a, np.float32) for a in (x, Wx0, Wx, Wh, bh, Wo, bo)]
    in_maps = [_pack_core(c, *args, NB, NSLOT) for c in range(N_CORES)]
    res = run_bass_kernel_spmd(nc, in_maps, list(range(N_CORES)),
                               trace=_trace)
    kernel.last_results = res
    return res.results[3]["out"]


if __name__ == "__main__":
    Tt = int(sys.argv[1]) if len(sys.argv) > 1 else 32
    rng = np.random.default_rng(0)
    STDV = 1.0 / np.sqrt(H)
    u = lambda *s: rng.uniform(-STDV, STDV, s).astype(np.float32)
    x = rng.standard_normal((B, Tt, I), dtype=np.float32)
    Wx0, Wx_, Wh_ = u(I, H), u(L - 1, H, H), u(L, H, H)
    bh_, Wo_, bo_ = u(L, H), u(H, C), u(C)

    h = np.zeros((L, B, H), np.float32)
    for t in range(Tt):
        inp = x[:, t, :]
        for l in range(L):
            pre = inp @ (Wx0 if l == 0 else Wx_[l - 1]) + h[l] @ Wh_[l] + bh_[l]
            h[l] = np.tanh(pre)
            inp = h[l]
    expected = h[-1] @ Wo_ + bo_

    got = kernel(x, Wx0, Wx_, Wh_, bh_, Wo_, bo_)
    err = np.abs(got - expected).max() / np.abs(expected).max()
    print(f"T={Tt}  max-rel-err: {err:.3e}")


# revision 5
# speedup vs baseline: 1.4001x; 1.0186x over previous
"""Trainium2 Bass kernel for nn_DeepRNN — V6: V5 + early half-copy of h^T.

Same 4-stage layer pipeline as V2 (2 groups of 4 cores, AllGather ring,
2-slot skew, per-core data for weights/masks/gates), plus:
  - all matmuls in bf16 (weights, h, x); PSUM accumulation stays fp32
  - x is zero-padded to H and merged into the gathered h via a 5th fused
    mask-add, so the input GEMM is always exactly 8 K-chunks
  - the per-step bias matmul is gone: bias is added by the DVE copy that
    moves GEMM PSUM tiles into the xpart buffer
  - GEMM tiles are emitted interleaved two steps ahead of the recurrence
    steps (G0 G1 R0 G2 R1 ...), so the PE fills the tanh-latency gaps of
    the recurrence chain with GEMM work
  - the h-input (msum) for slot s+1 is computed at the tail of slot s
"""
import sys
sys.path.insert(0, "/opt/trn_rl_repo")

import numpy as np
import ml_dtypes
import concourse.bacc as bacc
import concourse.mybir as mybir
import concourse.tile as tile
from concourse.bass_utils import run_bass_kernel_spmd

FP32 = mybir.dt.float32
BF16 = mybir.dt.bfloat16
TANH = mybir.ActivationFunctionType.Tanh
COPY = mybir.ActivationFunctionType.Copy
ADD = mybir.AluOpType.add
MULT = mybir.AluOpType.mult
BYPASS = mybir.AluOpType.bypass

N_CORES = 8
I, H, L, C, B, T = 256, 1024, 4, 1000, 64, 256
HC = H // 128
IC = I // 128
BLK = 8
SKEW = 2
GROUPS_A = [[0, 1], [2, 3], [4, 5], [6, 7]]
GROUPS_B = [[0, 3], [1, 2], [4, 7], [5, 6]]


def build(T_steps=T):
    nc = bacc.Bacc(num_devices=N_CORES)
    NB = T_steps // BLK
    NSLOT = NB + SKEW * (L - 1)

    Wh = nc.declare_dram_parameter("Wh", [128, HC, H], BF16, isOutput=False)
    Wg = nc.declare_dram_parameter("Wg", [128, HC, H], BF16, isOutput=False)
    xT = nc.declare_dram_parameter("xT", [128, NB, BLK, IC, B], BF16,
                                   isOutput=False)
    hmask = nc.declare_dram_parameter("hmask", [128, 3], FP32, isOutput=False)
    gates = nc.declare_dram_parameter("gates", [128, NSLOT], FP32,
                                      isOutput=False)
    identb = nc.declare_dram_parameter("identb", [128, B], BF16,
                                       isOutput=False)
    onesg = nc.declare_dram_parameter("onesg", [128, B], BF16, isOutput=False)
    biasb = nc.declare_dram_parameter("biasb", [64, H], FP32, isOutput=False)
    hzero = nc.declare_dram_parameter("hzero", [128, HC, B], BF16,
                                      isOutput=False)
    Wo_p = nc.declare_dram_parameter("Wo", [128, HC, C], BF16, isOutput=False)
    Wob = nc.declare_dram_parameter("Wob", [128, C], BF16, isOutput=False)
    out = nc.declare_dram_parameter("out", [B, C], FP32, isOutput=True)

    with tile.TileContext(nc) as tc:
        with (
            tc.tile_pool(name="wpool", bufs=1) as wpool,
            tc.tile_pool(name="cpool", bufs=1) as cpool,
            tc.tile_pool(name="xpool", bufs=2) as xpool,
            tc.tile_pool(name="agpool", bufs=1) as agpool,
            tc.tile_pool(name="tpool", bufs=1) as tpool,
            tc.tile_pool(name="hpool", bufs=2) as hpool,
            tc.tile_pool(name="xppool", bufs=1) as xppool,
            tc.tile_pool(name="stpool", bufs=2) as stpool,
            tc.tile_pool(name="htpool", bufs=2) as htpool,
            tc.tile_pool(name="hspool", bufs=2) as hspool,
            tc.tile_pool(name="ps_ab", bufs=2, space="PSUM") as ps_ab,
            tc.tile_pool(name="ps_tr", bufs=2, space="PSUM") as ps_tr,
            tc.tile_pool(name="dpool", bufs=1, space="DRAM") as dpool,
        ):
            wh_sb = wpool.tile([128, HC, H], BF16, tag="wh")
            nc.sync.dma_start(wh_sb[:], Wh[:])
            wg_sb = wpool.tile([128, HC, H], BF16, tag="wg")
            nc.sync.dma_start(wg_sb[:], Wg[:])
            mask_sb = cpool.tile([128, 3], FP32, tag="hm")
            nc.sync.dma_start(mask_sb[:], hmask[:])
            gates_sb = cpool.tile([128, NSLOT], FP32, tag="gt")
            nc.sync.dma_start(gates_sb[:], gates[:])
            identb_sb = cpool.tile([128, B], BF16, tag="id")
            nc.sync.dma_start(identb_sb[:], identb[:])
            onesg_sb = cpool.tile([128, B], BF16, tag="og")
            nc.sync.dma_start(onesg_sb[:], onesg[:])
            biasb_sb = cpool.tile([64, H], FP32, tag="bb")
            nc.sync.dma_start(biasb_sb[:], biasb[:])
            hT_init = cpool.tile([128, HC, B], BF16, tag="h0")
            nc.sync.dma_start(hT_init[:], hzero[:])

            bin_t = [dpool.tile([128, BLK, HC, B], BF16, tag=f"bi{i}",
                                name=f"bi{i}") for i in range(2)]
            boutA_t = [dpool.tile([2, 128, BLK, HC, B], BF16, tag=f"ba{i}",
                                  name=f"ba{i}") for i in range(2)]
            boutB_t = [dpool.tile([2, 128, BLK, HC, B], BF16, tag=f"bb{i}",
                                  name=f"bb{i}") for i in range(2)]

            def emit_hin(fs):
                """Build hin for slot fs (call at tail of slot fs-1).

                Returns (hin_tile, n_chunks): n_chunks = IC during warmup
                slots (x only), HC afterwards."""
                xbt = xpool.tile([128, BLK, IC, B], BF16, tag="xb",
                                 name="xbt")
                nc.sync.dma_start(xbt[:], xT[:, min(fs, NB - 1), :, :, :])
                if fs < SKEW:
                    hin_t = tpool.tile([128, BLK, IC, B], BF16, tag="hinx",
                                       name="hin_x")
                    nc.vector.tensor_scalar_mul(hin_t[:], xbt[:],
                                                mask_sb[:, 2:3])
                    return hin_t, IC
                hin_t = hpool.tile([128, BLK, HC, B], BF16, tag="hin",
                                   name="hin_t")
                agA = agpool.tile([128, BLK, HC, B], BF16, tag="agA",
                                  name="agA")
                nc.sync.dma_start(agA[:], boutA_t[(fs - SKEW) % 2][0])
                agB = agpool.tile([128, BLK, HC, B], BF16, tag="agB",
                                  name="agB")
                nc.sync.dma_start(agB[:], boutB_t[(fs - SKEW) % 2][0])
                acc = tpool.tile([128, BLK, HC, B], BF16, tag="accA",
                                 name="accA")
                nc.vector.tensor_scalar_mul(acc[:], agA[:], mask_sb[:, 0:1])
                nc.vector.scalar_tensor_tensor(
                    hin_t[:], agB[:], mask_sb[:, 1:2], acc[:],
                    op0=MULT, op1=ADD)
                nc.vector.scalar_tensor_tensor(
                    hin_t[:, :, 0:IC, :], xbt[:], mask_sb[:, 2:3],
                    hin_t[:, :, 0:IC, :], op0=MULT, op1=ADD)
                return hin_t, HC

            def emit_gemm_tile(hin_t, nch, xpart, t):
                gA = ps_ab.tile([64, 512], FP32, tag="pA", name="gA")
                gB = ps_ab.tile([64, 512], FP32, tag="pB", name="gB")
                for k in range(nch):
                    st, sp = k == 0, k == nch - 1
                    nc.tensor.matmul(gA[:], hin_t[:, t, k, :],
                                     wg_sb[:, k, 0:512], start=st, stop=sp)
                    nc.tensor.matmul(gB[:], hin_t[:, t, k, :],
                                     wg_sb[:, k, 512:H], start=st, stop=sp)
                nc.vector.tensor_tensor(xpart[:, t, 0:512], gA[:],
                                        biasb_sb[:, 0:512], op=ADD)
                nc.vector.tensor_tensor(xpart[:, t, 512:H], gB[:],
                                        biasb_sb[:, 512:H], op=ADD)

            hin_t, nch = emit_hin(0)
            stage_prev = None
            stage = None
            for s in range(NSLOT):
                xpart = xppool.tile([64, BLK, H], BF16, tag="xp")
                stage = stpool.tile([128, BLK, HC, B], BF16, tag="st",
                                    name="stage")
                hT_g = htpool.tile([128, HC, B], BF16, tag="hg")
                if stage_prev is None:
                    nc.vector.tensor_scalar_mul(
                        hT_g[:], hT_init[:], gates_sb[:, s:s + 1])
                else:
                    nc.vector.tensor_scalar_mul(
                        hT_g[:], stage_prev[:, BLK - 1, :, :],
                        gates_sb[:, s:s + 1])
                hT_prev = hT_g

                emit_gemm_tile(hin_t, nch, xpart, 0)
                emit_gemm_tile(hin_t, nch, xpart, 1)
                for t in range(BLK):
                    # ---- recurrence step t ----
                    pA = ps_ab.tile([64, 512], FP32, tag="pA", name="pA")
                    pB = ps_ab.tile([64, 512], FP32, tag="pB", name="pB")
                    nc.scalar.activation(pA[:], xpart[:, t, 0:512], COPY)
                    nc.scalar.activation(pB[:], xpart[:, t, 512:H], COPY)
                    for k in range(HC):
                        nc.tensor.matmul(pA[:], hT_prev[:, k, :],
                                         wh_sb[:, k, 0:512], start=False,
                                         stop=k == HC - 1,
                                         skip_group_check=True)
                    h_sb = hspool.tile([64, H], BF16, tag="h")
                    nc.scalar.activation(h_sb[:, 0:512], pA[:], TANH)
                    for k in range(HC):
                        nc.tensor.matmul(pB[:], hT_prev[:, k, :],
                                         wh_sb[:, k, 512:H], start=False,
                                         stop=k == HC - 1,
                                         skip_group_check=True)
                    pt = ps_tr.tile([128, HC, B], BF16, tag="pt")
                    for k in range(4):
                        nc.tensor.transpose(pt[:, k, :],
                                            h_sb[:, k * 128:(k + 1) * 128],
                                            identb_sb[0:64, 0:64])
                    nc.scalar.activation(h_sb[:, 512:H], pB[:], TANH)
                    nc.vector.tensor_copy(stage[:, t, 0:4, :], pt[:, 0:4, :])
                    for k in range(4, HC):
                        nc.tensor.transpose(pt[:, k, :],
                                            h_sb[:, k * 128:(k + 1) * 128],
                                            identb_sb[0:64, 0:64])
                    nc.vector.tensor_copy(stage[:, t, 4:HC, :],
                                          pt[:, 4:HC, :])
                    hT_prev = stage[:, t, :, :]
                    # ---- interleave: GEMM tile t+2 of this slot ----
                    if t + 2 < BLK:
                        emit_gemm_tile(hin_t, nch, xpart, t + 2)

                # ---- publish h block, prefetch next hin ----
                bin_ = bin_t[s % 2]
                nc.sync.dma_start(bin_[:], stage[:])
                nc.gpsimd.collective_compute(
                    "AllGather", BYPASS, replica_groups=GROUPS_A,
                    ins=[bin_[:].opt()], outs=[boutA_t[s % 2][:].opt()])
                nc.gpsimd.collective_compute(
                    "AllGather", BYPASS, replica_groups=GROUPS_B,
                    ins=[bin_[:].opt()], outs=[boutB_t[s % 2][:].opt()])
                if s + 1 < NSLOT:
                    hin_t, nch = emit_hin(s + 1)
                stage_prev = stage

            # ---- classifier head (bf16) ----
            wo_sb = wpool.tile([128, HC, C], BF16, tag="wg")
            nc.sync.dma_start(wo_sb[:], Wo_p[:])
            wob_sb = cpool.tile([128, C], BF16, tag="wob")
            nc.sync.dma_start(wob_sb[:], Wob[:])

            hA = ps_ab.tile([64, 512], FP32, tag="pA", name="hA")
            hB = ps_ab.tile([64, 512], FP32, tag="pB", name="hB")
            hT_bf = stage[:, BLK - 1, :, :]
            for k in range(HC + 1):
                st, sp = k == 0, k == HC
                if k == HC:
                    nc.tensor.matmul(hA[:], onesg_sb[0:1, :],
                                     wob_sb[0:1, 0:512], start=st, stop=sp)
                    nc.tensor.matmul(hB[:, 0:C - 512], onesg_sb[0:1, :],
                                     wob_sb[0:1, 512:C], start=st, stop=sp)
                else:
                    nc.tensor.matmul(hA[:], hT_bf[:, k, :],
                                     wo_sb[:, k, 0:512], start=st, stop=sp)
                    nc.tensor.matmul(hB[:, 0:C - 512], hT_bf[:, k, :],
                                     wo_sb[:, k, 512:C], start=st, stop=sp)
            out_sb = hspool.tile([64, C], FP32, tag="osb")
            nc.scalar.activation(out_sb[:, 0:512], hA[:], COPY)
            nc.scalar.activation(out_sb[:, 512:C], hB[:, 0:C - 512], COPY)
            nc.sync.dma_start(out[:], out_sb[:])

    nc.compile()
    return nc, NSLOT


def _to_bf16(a):
    return np.asarray(a, dtype=ml_dtypes.bfloat16)


def _pack_core(c, x, Wx0, Wx, Wh_, bh, Wo, bo, NB, NSLOT):
    l = c % 4
    f32 = np.float32
    wh = np.ascontiguousarray(
        Wh_[l].reshape(HC, 128, H).transpose(1, 0, 2), dtype=f32)

    wg = np.zeros((128, HC, H), f32)
    if l > 0:
        wg[:, :, :] = Wx[l - 1].reshape(HC, 128, H).transpose(1, 0, 2)
    else:
        wg[:, 0:IC, :] = Wx0.reshape(IC, 128, H).transpose(1, 0, 2)

    # xT[p, blk, t8, c, b] = x[b, blk*BLK+t8, c*128+p]
    xs = x.transpose(2, 1, 0).reshape(IC, 128, NB, BLK, B)
    xt = np.ascontiguousarray(xs.transpose(1, 2, 3, 0, 4), dtype=f32)

    # hmask = [use-A-gather, use-B-gather, x-merge]
    hm = np.zeros((128, 3), f32)
    if l in (1, 3):
        hm[:, 0] = 1.0
    elif l == 2:
        hm[:, 1] = 1.0
    else:
        hm[:, 2] = 1.0

    gt = np.zeros((128, NSLOT), f32)
    gt[:, :] = (np.arange(NSLOT) > 2 * l).astype(f32)[None, :]

    eye = np.eye(64, dtype=f32)
    ones = np.zeros((128, B), f32)
    ones[0] = 1.0
    wob = np.zeros((128, C), f32)
    wob[0] = bo

    return {
        "Wh": _to_bf16(wh),
        "Wg": _to_bf16(wg),
        "xT": _to_bf16(xt),
        "hmask": hm,
        "gates": gt,
        "identb": _to_bf16(np.vstack([eye, eye])),
        "onesg": _to_bf16(ones),
        "biasb": np.tile(bh[l][None, :], (64, 1)).astype(f32),
        "hzero": _to_bf16(np.zeros((128, HC, B), f32)),
        "Wo": _to_bf16(Wo.reshape(HC, 128, C).transpose(1, 0, 2)),
        "Wob": _to_bf16(wob),
    }


_BUILT = {}


def kernel(x, Wx0, Wx, Wh, bh, Wo, bo, _trace=False):
    T_steps = x.shape[1]
    NB = T_steps // BLK
    if T_steps not in _BUILT:
        _BUILT[T_steps] = build(T_steps)
    nc, NSLOT = _BUILT[T_steps]
    args = [np.asarray(a, np.float32) for a in (x, Wx0, Wx, Wh, bh, Wo, bo)]
    in_maps = [_pack_core(c, *args, NB, NSLOT) for c in range(N_CORES)]
    res = run_bass_kernel_spmd(nc, in_maps, list(range(N_CORES)),
                               trace=_trace)
    kernel.last_results = res
    return res.results[3]["out"]


if __name__ == "__main__":
    Tt = int(sys.argv[1]) if len(sys.argv) > 1 else 32
    rng = np.random.default_rng(0)
    STDV = 1.0 / np.sqrt(H)
    u = lambda *s: rng.uniform(-STDV, STDV, s).astype(np.float32)
    x = rng.standard_normal((B, Tt, I), dtype=np.float32)
    Wx0, Wx_, Wh_ = u(I, H), u(L - 1, H, H), u(L, H, H)
    bh_, Wo_, bo_ = u(L, H), u(H, C), u(C)

    h = np.zeros((L, B, H), np.float32)
    for t in range(Tt):
        inp = x[:, t, :]
        for l in range(L):
            pre = inp @ (Wx0 if l == 0 else Wx_[l - 1]) + h[l] @ Wh_[l] + bh_[l]
            h[l] = np.tanh(pre)
            inp = h[l]
    expected = h[-1] @ Wo_ + bo_

    got = kernel(x, Wx0, Wx_, Wh_, bh_, Wo_, bo_)
    err = np.abs(got - expected).max() / np.abs(expected).max()
    print(f"T={Tt}  max-rel-err: {err:.3e}")


# revision 6
# speedup vs baseline: 1.4043x; 1.0030x over previous
"""Trainium2 Bass kernel for nn_DeepRNN — V7: mid-slot hin prefetch.

Same 4-stage layer pipeline as V2 (2 groups of 4 cores, AllGather ring,
2-slot skew, per-core data for weights/masks/gates), plus:
  - all matmuls in bf16 (weights, h, x); PSUM accumulation stays fp32
  - x is zero-padded to H and merged into the gathered h via a 5th fused
    mask-add, so the input GEMM is always exactly 8 K-chunks
  - the per-step bias matmul is gone: bias is added by the DVE copy that
    moves GEMM PSUM tiles into the xpart buffer
  - GEMM tiles are emitted interleaved two steps ahead of the recurrence
    steps (G0 G1 R0 G2 R1 ...), so the PE fills the tanh-latency gaps of
    the recurrence chain with GEMM work
  - the h-input (msum) for slot s+1 is computed at the tail of slot s
"""
import sys
sys.path.insert(0, "/opt/trn_rl_repo")

import numpy as np
import ml_dtypes
import concourse.bacc as bacc
import concourse.mybir as mybir
import concourse.tile as tile
from concourse.bass_utils import run_bass_kernel_spmd

FP32 = mybir.dt.float32
BF16 = mybir.dt.bfloat16
TANH = mybir.ActivationFunctionType.Tanh
COPY = mybir.ActivationFunctionType.Copy
ADD = mybir.AluOpType.add
MULT = mybir.AluOpType.mult
BYPASS = mybir.AluOpType.bypass

N_CORES = 8
I, H, L, C, B, T = 256, 1024, 4, 1000, 64, 256
HC = H // 128
IC = I // 128
BLK = 8
SKEW = 2
GROUPS_A = [[0, 1], [2, 3], [4, 5], [6, 7]]
GROUPS_B = [[0, 3], [1, 2], [4, 7], [5, 6]]


def build(T_steps=T):
    nc = bacc.Bacc(num_devices=N_CORES)
    NB = T_steps // BLK
    NSLOT = NB + SKEW * (L - 1)

    Wh = nc.declare_dram_parameter("Wh", [128, HC, H], BF16, isOutput=False)
    Wg = nc.declare_dram_parameter("Wg", [128, HC, H], BF16, isOutput=False)
    xT = nc.declare_dram_parameter("xT", [128, NB, BLK, IC, B], BF16,
                                   isOutput=False)
    hmask = nc.declare_dram_parameter("hmask", [128, 3], FP32, isOutput=False)
    gates = nc.declare_dram_parameter("gates", [128, NSLOT], FP32,
                                      isOutput=False)
    identb = nc.declare_dram_parameter("identb", [128, B], BF16,
                                       isOutput=False)
    onesg = nc.declare_dram_parameter("onesg", [128, B], BF16, isOutput=False)
    biasb = nc.declare_dram_parameter("biasb", [64, H], FP32, isOutput=False)
    hzero = nc.declare_dram_parameter("hzero", [128, HC, B], BF16,
                                      isOutput=False)
    Wo_p = nc.declare_dram_parameter("Wo", [128, HC, C], BF16, isOutput=False)
    Wob = nc.declare_dram_parameter("Wob", [128, C], BF16, isOutput=False)
    out = nc.declare_dram_parameter("out", [B, C], FP32, isOutput=True)

    with tile.TileContext(nc) as tc:
        with (
            tc.tile_pool(name="wpool", bufs=1) as wpool,
            tc.tile_pool(name="cpool", bufs=1) as cpool,
            tc.tile_pool(name="xpool", bufs=2) as xpool,
            tc.tile_pool(name="agpool", bufs=1) as agpool,
            tc.tile_pool(name="tpool", bufs=1) as tpool,
            tc.tile_pool(name="hpool", bufs=2) as hpool,
            tc.tile_pool(name="xppool", bufs=1) as xppool,
            tc.tile_pool(name="stpool", bufs=2) as stpool,
            tc.tile_pool(name="htpool", bufs=2) as htpool,
            tc.tile_pool(name="hspool", bufs=2) as hspool,
            tc.tile_pool(name="ps_ab", bufs=2, space="PSUM") as ps_ab,
            tc.tile_pool(name="ps_tr", bufs=2, space="PSUM") as ps_tr,
            tc.tile_pool(name="dpool", bufs=1, space="DRAM") as dpool,
        ):
            wh_sb = wpool.tile([128, HC, H], BF16, tag="wh")
            nc.sync.dma_start(wh_sb[:], Wh[:])
            wg_sb = wpool.tile([128, HC, H], BF16, tag="wg")
            nc.sync.dma_start(wg_sb[:], Wg[:])
            mask_sb = cpool.tile([128, 3], FP32, tag="hm")
            nc.sync.dma_start(mask_sb[:], hmask[:])
            gates_sb = cpool.tile([128, NSLOT], FP32, tag="gt")
            nc.sync.dma_start(gates_sb[:], gates[:])
            identb_sb = cpool.tile([128, B], BF16, tag="id")
            nc.sync.dma_start(identb_sb[:], identb[:])
            onesg_sb = cpool.tile([128, B], BF16, tag="og")
            nc.sync.dma_start(onesg_sb[:], onesg[:])
            biasb_sb = cpool.tile([64, H], FP32, tag="bb")
            nc.sync.dma_start(biasb_sb[:], biasb[:])
            hT_init = cpool.tile([128, HC, B], BF16, tag="h0")
            nc.sync.dma_start(hT_init[:], hzero[:])

            bin_t = [dpool.tile([128, BLK, HC, B], BF16, tag=f"bi{i}",
                                name=f"bi{i}") for i in range(2)]
            boutA_t = [dpool.tile([2, 128, BLK, HC, B], BF16, tag=f"ba{i}",
                                  name=f"ba{i}") for i in range(2)]
            boutB_t = [dpool.tile([2, 128, BLK, HC, B], BF16, tag=f"bb{i}",
                                  name=f"bb{i}") for i in range(2)]

            def emit_hin(fs):
                """Prefetch + build hin for slot fs.

                Issues the DMAs immediately (scalar-engine DGE queue) and
                returns (hin_tile, n_chunks, [op closures]) — the closures
                emit the DVE mask-merge ops and are interleaved at the
                recurrence step tails by the caller."""
                xbt = xpool.tile([128, BLK, IC, B], BF16, tag="xb",
                                 name="xbt")
                nc.scalar.dma_start(xbt[:], xT[:, min(fs, NB - 1), :, :, :])
                if fs < SKEW:
                    hin_t = tpool.tile([128, BLK, IC, B], BF16, tag="hinx",
                                       name="hin_x")
                    nc.vector.tensor_scalar_mul(hin_t[:], xbt[:],
                                                mask_sb[:, 2:3])
                    return hin_t, IC, []
                hin_t = hpool.tile([128, BLK, HC, B], BF16, tag="hin",
                                   name="hin_t")
                agA = agpool.tile([128, BLK, HC, B], BF16, tag="agA",
                                  name="agA")
                nc.scalar.dma_start(agA[:], boutA_t[(fs - SKEW) % 2][0])
                agB = agpool.tile([128, BLK, HC, B], BF16, tag="agB",
                                  name="agB")
                nc.scalar.dma_start(agB[:], boutB_t[(fs - SKEW) % 2][0])
                acc = tpool.tile([128, BLK, HC, B], BF16, tag="accA",
                                 name="accA")
                ops = [
                    lambda: nc.vector.tensor_scalar_mul(
                        acc[:], agA[:], mask_sb[:, 0:1]),
                    lambda: nc.vector.scalar_tensor_tensor(
                        hin_t[:], agB[:], mask_sb[:, 1:2], acc[:],
                        op0=MULT, op1=ADD),
                    lambda: nc.vector.scalar_tensor_tensor(
                        hin_t[:, :, 0:IC, :], xbt[:], mask_sb[:, 2:3],
                        hin_t[:, :, 0:IC, :], op0=MULT, op1=ADD),
                ]
                return hin_t, HC, ops

            def emit_gemm_tile(hin_t, nch, xpart, t):
                gA = ps_ab.tile([64, 512], FP32, tag="pA", name="gA")
                gB = ps_ab.tile([64, 512], FP32, tag="pB", name="gB")
                for k in range(nch):
                    st, sp = k == 0, k == nch - 1
                    nc.tensor.matmul(gA[:], hin_t[:, t, k, :],
                                     wg_sb[:, k, 0:512], start=st, stop=sp)
                    nc.tensor.matmul(gB[:], hin_t[:, t, k, :],
                                     wg_sb[:, k, 512:H], start=st, stop=sp)
                nc.vector.tensor_tensor(xpart[:, t, 0:512], gA[:],
                                        biasb_sb[:, 0:512], op=ADD)
                nc.vector.tensor_tensor(xpart[:, t, 512:H], gB[:],
                                        biasb_sb[:, 512:H], op=ADD)

            hin_t, nch, hin_ops = emit_hin(0)
            for op in hin_ops:
                op()
            nxt = None
            stage_prev = None
            stage = None
            for s in range(NSLOT):
                xpart = xppool.tile([64, BLK, H], BF16, tag="xp")
                stage = stpool.tile([128, BLK, HC, B], BF16, tag="st",
                                    name="stage")
                if stage_prev is None:
                    hT_g = htpool.tile([128, HC, B], BF16, tag="hg")
                    nc.vector.tensor_scalar_mul(
                        hT_g[:], hT_init[:], gates_sb[:, s:s + 1])
                hT_prev = hT_g

                emit_gemm_tile(hin_t, nch, xpart, 0)
                emit_gemm_tile(hin_t, nch, xpart, 1)
                for t in range(BLK):
                    # ---- recurrence step t ----
                    pA = ps_ab.tile([64, 512], FP32, tag="pA", name="pA")
                    pB = ps_ab.tile([64, 512], FP32, tag="pB", name="pB")
                    nc.scalar.activation(pA[:], xpart[:, t, 0:512], COPY)
                    nc.scalar.activation(pB[:], xpart[:, t, 512:H], COPY)
                    for k in range(HC):
                        nc.tensor.matmul(pA[:], hT_prev[:, k, :],
                                         wh_sb[:, k, 0:512], start=False,
                                         stop=k == HC - 1,
                                         skip_group_check=True)
                    h_sb = hspool.tile([64, H], BF16, tag="h")
                    nc.scalar.activation(h_sb[:, 0:512], pA[:], TANH)
                    for k in range(HC):
                        nc.tensor.matmul(pB[:], hT_prev[:, k, :],
                                         wh_sb[:, k, 512:H], start=False,
                                         stop=k == HC - 1,
                                         skip_group_check=True)
                    pt = ps_tr.tile([128, HC, B], BF16, tag="pt")
                    for k in range(4):
                        nc.tensor.transpose(pt[:, k, :],
                                            h_sb[:, k * 128:(k + 1) * 128],
                                            identb_sb[0:64, 0:64])
                    nc.scalar.activation(h_sb[:, 512:H], pB[:], TANH)
                    nc.vector.tensor_copy(stage[:, t, 0:4, :], pt[:, 0:4, :])
                    # ---- interleave: GEMM tile t+2 (fills tanh-B wait) ----
                    if t + 2 < BLK:
                        emit_gemm_tile(hin_t, nch, xpart, t + 2)
                    for k in range(4, HC):
                        nc.tensor.transpose(pt[:, k, :],
                                            h_sb[:, k * 128:(k + 1) * 128],
                                            identb_sb[0:64, 0:64])
                    nc.vector.tensor_copy(stage[:, t, 4:HC, :],
                                          pt[:, 4:HC, :])
                    hT_prev = stage[:, t, :, :]
                    # ---- prefetch next slot's hin; spread its DVE ops ----
                    if t == 2 and s + 1 < NSLOT:
                        nxt = emit_hin(s + 1)
                    if nxt is not None and t >= 3 and nxt[2]:
                        nxt[2].pop(0)()

                # ---- publish h block, prefetch next hin ----
                # remaining hin ops, next slot's carry gate
                if nxt is not None:
                    for op in nxt[2]:
                        op()
                    nxt[2].clear()
                if s + 1 < NSLOT:
                    hT_g = htpool.tile([128, HC, B], BF16, tag="hg")
                    nc.vector.tensor_scalar_mul(
                        hT_g[:], stage[:, BLK - 1, :, :],
                        gates_sb[:, s + 1:s + 2])
                bin_ = bin_t[s % 2]
                nc.sync.dma_start(bin_[:], stage[:])
                nc.gpsimd.collective_compute(
                    "AllGather", BYPASS, replica_groups=GROUPS_A,
                    ins=[bin_[:].opt()], outs=[boutA_t[s % 2][:].opt()])
                nc.gpsimd.collective_compute(
                    "AllGather", BYPASS, replica_groups=GROUPS_B,
                    ins=[bin_[:].opt()], outs=[boutB_t[s % 2][:].opt()])
                if nxt is not None:
                    hin_t, nch = nxt[0], nxt[1]
                    nxt = None
                stage_prev = stage

            # ---- classifier head (bf16) ----
            wo_sb = wpool.tile([128, HC, C], BF16, tag="wg")
            nc.sync.dma_start(wo_sb[:], Wo_p[:])
            wob_sb = cpool.tile([128, C], BF16, tag="wob")
            nc.sync.dma_start(wob_sb[:], Wob[:])

            hA = ps_ab.tile([64, 512], FP32, tag="pA", name="hA")
            hB = ps_ab.tile([64, 512], FP32, tag="pB", name="hB")
            hT_bf = stage[:, BLK - 1, :, :]
            for k in range(HC + 1):
                st, sp = k == 0, k == HC
                if k == HC:
                    nc.tensor.matmul(hA[:], onesg_sb[0:1, :],
                                     wob_sb[0:1, 0:512], start=st, stop=sp)
                    nc.tensor.matmul(hB[:, 0:C - 512], onesg_sb[0:1, :],
                                     wob_sb[0:1, 512:C], start=st, stop=sp)
                else:
                    nc.tensor.matmul(hA[:], hT_bf[:, k, :],
                                     wo_sb[:, k, 0:512], start=st, stop=sp)
                    nc.tensor.matmul(hB[:, 0:C - 512], hT_bf[:, k, :],
                                     wo_sb[:, k, 512:C], start=st, stop=sp)
            out_sb = hspool.tile([64, C], FP32, tag="osb")
            nc.scalar.activation(out_sb[:, 0:512], hA[:], COPY)
            nc.scalar.activation(out_sb[:, 512:C], hB[:, 0:C - 512], COPY)
            nc.sync.dma_start(out[:], out_sb[:])

    nc.compile()
    return nc, NSLOT


def _to_bf16(a):
    return np.asarray(a, dtype=ml_dtypes.bfloat16)


def _pack_core(c, x, Wx0, Wx, Wh_, bh, Wo, bo, NB, NSLOT):
    l = c % 4
    f32 = np.float32
    wh = np.ascontiguousarray(
        Wh_[l].reshape(HC, 128, H).transpose(1, 0, 2), dtype=f32)

    wg = np.zeros((128, HC, H), f32)
    if l > 0:
        wg[:, :, :] = Wx[l - 1].reshape(HC, 128, H).transpose(1, 0, 2)
    else:
        wg[:, 0:IC, :] = Wx0.reshape(IC, 128, H).transpose(1, 0, 2)

    # xT[p, blk, t8, c, b] = x[b, blk*BLK+t8, c*128+p]
    xs = x.transpose(2, 1, 0).reshape(IC, 128, NB, BLK, B)
    xt = np.ascontiguousarray(xs.transpose(1, 2, 3, 0, 4), dtype=f32)

    # hmask = [use-A-gather, use-B-gather, x-merge]
    hm = np.zeros((128, 3), f32)
    if l in (1, 3):
        hm[:, 0] = 1.0
    elif l == 2:
        hm[:, 1] = 1.0
    else:
        hm[:, 2] = 1.0

    gt = np.zeros((128, NSLOT), f32)
    gt[:, :] = (np.arange(NSLOT) > 2 * l).astype(f32)[None, :]

    eye = np.eye(64, dtype=f32)
    ones = np.zeros((128, B), f32)
    ones[0] = 1.0
    wob = np.zeros((128, C), f32)
    wob[0] = bo

    return {
        "Wh": _to_bf16(wh),
        "Wg": _to_bf16(wg),
        "xT": _to_bf16(xt),
        "hmask": hm,
        "gates": gt,
        "identb": _to_bf16(np.vstack([eye, eye])),
        "onesg": _to_bf16(ones),
        "biasb": np.tile(bh[l][None, :], (64, 1)).astype(f32),
        "hzero": _to_bf16(np.zeros((128, HC, B), f32)),
        "Wo": _to_bf16(Wo.reshape(HC, 128, C).transpose(1, 0, 2)),
        "Wob": _to_bf16(wob),
    }


_BUILT = {}


def kernel(x, Wx0, Wx, Wh, bh, Wo, bo, _trace=False):
    T_steps = x.shape[1]
    NB = T_steps // BLK
    if T_steps not in _BUILT:
        _BUILT[T_steps] = build(T_steps)
    nc, NSLOT = _BUILT[T_steps]
    args = [np.asarray(a, np.float32) for a in (x, Wx0, Wx, Wh, bh, Wo, bo)]
    in_maps = [_pack_core(c, *args, NB, NSLOT) for c in range(N_CORES)]
    res = run_bass_kernel_spmd(nc, in_maps, list(range(N_CORES)),
                               trace=_trace)
    kernel.last_results = res
    return res.results[3]["out"]


if __name__ == "__main__":
    Tt = int(sys.argv[1]) if len(sys.argv) > 1 else 32
    rng = np.random.default_rng(0)
    STDV = 1.0 / np.sqrt(H)
    u = lambda *s: rng.uniform(-STDV, STDV, s).astype(np.float32)
    x = rng.standard_normal((B, Tt, I), dtype=np.float32)
    Wx0, Wx_, Wh_ = u(I, H), u(L - 1, H, H), u(L, H, H)
    bh_, Wo_, bo_ = u(L, H), u(H, C), u(C)

    h = np.zeros((L, B, H), np.float32)
    for t in range(Tt):
        inp = x[:, t, :]
        for l in range(L):
            pre = inp @ (Wx0 if l == 0 else Wx_[l - 1]) + h[l] @ Wh_[l] + bh_[l]
            h[l] = np.tanh(pre)
            inp = h[l]
    expected = h[-1] @ Wo_ + bo_

    got = kernel(x, Wx0, Wx_, Wh_, bh_, Wo_, bo_)
    err = np.abs(got - expected).max() / np.abs(expected).max()
    print(f"T={Tt}  max-rel-err: {err:.3e}")
